# revision 10
# baseline (speedup 1.0000x reference)
"""Trainium2 Bass kernel for nn_AladynoulliModel (N=5000, K=20, T=100, D=128).

  theta    = softmax(lambda_, axis=1)            [N,K,T]
  phi_prob = sigmoid(phi)                        [K,D,T]
  pi       = einsum('nkt,kdt->ndt', theta, phi_prob)
  pi       = clip(pi * (t <= event_times[n,d]), EPS, 1-EPS)

Sharding: data-parallel over N across 8 cores (625 rows each); phi replicated.

Per-core pipeline, per 128-row n-tile:
  - softmax over K in fp32 (n on partitions); theta output stays fp32
  - theta cast to bf16, transposed via PE matmul-against-identity with
    tile_position column placement: t-block (t%4)*32 psum quadrant rows
  - pi matmuls in 32x128 row-tiled PE mode (bf16 in, fp32 psum): quadrant
    q = t%4 contracts k=20 against phi_prob bf16 replicated in SBUF quadrants
  - fused DVE pass: pi = min(psum, 1-EPS) * mask; mask = (t <= ET) built on
    GPSIMD in bf16 via broadcast APs; eps-floor pass; contiguous DMA out
"""
import sys

sys.path.insert(0, "/opt/trn_rl_repo")

import numpy as np
from contextlib import ExitStack

import concourse.bacc as bacc
import concourse.bass as bass
import concourse.mybir as mybir
import concourse.tile as tile
from concourse import masks
from concourse.bass_utils import run_bass_kernel_spmd

N, K, T, D = 5000, 20, 100, 128
KP = 32                      # k padded to PE quadrant stride
NCORES = 8
NS = N // NCORES             # 625 rows per core
EPS = 1e-8
FP32 = mybir.dt.float32
BF16 = mybir.dt.bfloat16
AF = mybir.ActivationFunctionType
OP = mybir.AluOpType

ROW_TILES = [(i * 128, min(128, NS - i * 128)) for i in range((NS + 127) // 128)]

LAST_RESULTS = None          # test harness reads exec_time_ns off this


def _build_nc():
    nc = bacc.Bacc("TRN2", target_bir_lowering=False, debug=False)
    lam = nc.dram_tensor("lam", [NS, K * T], FP32, kind="ExternalInput")
    phi = nc.dram_tensor("phi", [K, D * T], FP32, kind="ExternalInput")
    etf = nc.dram_tensor("etf", [NS, D], FP32, kind="ExternalInput")
    pi_o = nc.dram_tensor("pi", [NS, D * T], FP32, kind="ExternalOutput")
    th_o = nc.dram_tensor("theta", [NS, K * T], FP32, kind="ExternalOutput")
    pp_o = nc.dram_tensor("phiprob", [K, D * T], FP32, kind="ExternalOutput")

    with tile.TileContext(nc) as tc, ExitStack() as ctx:
        const_pool = ctx.enter_context(tc.tile_pool(name="const", bufs=1))
        big_pool = ctx.enter_context(tc.tile_pool(name="big", bufs=1))
        small_pool = ctx.enter_context(tc.tile_pool(name="small", bufs=2))
        psum_t = ctx.enter_context(tc.tile_pool(name="psum_t", bufs=2, space="PSUM"))
        psum_m = ctx.enter_context(tc.tile_pool(name="psum_m", bufs=1, space="PSUM"))

        ident = const_pool.tile([128, 128], BF16)
        masks.make_identity(nc, ident[:])
        iota = const_pool.tile([128, T], FP32)
        nc.gpsimd.iota(iota[:], pattern=[[1, T]], base=0, channel_multiplier=0,
                       allow_small_or_imprecise_dtypes=True)

        # phi -> sigmoid (fp32, for output) -> bf16 quadrant replicas for matmul
        pp_f = const_pool.tile([K, D * T], FP32)
        nc.sync.dma_start(pp_f[:], phi[:])
        nc.scalar.activation(pp_f[:], pp_f[:], AF.Sigmoid)
        nc.sync.dma_start(pp_o[:], pp_f[:])
        pp_bf = const_pool.tile([128, D * T], BF16)
        nc.vector.tensor_copy(pp_bf[0:K, :], pp_f[:])
        for q in range(1, 4):
            nc.sync.dma_start(pp_bf[q * KP:q * KP + K, :], pp_bf[0:K, :])

        # persistent bf16 theta staging tile; pad cols zeroed once
        th_bf = const_pool.tile([128, KP * T], BF16)
        nc.gpsimd.memset(th_bf[:, K * T:], 0.0)

        NG = T // 4  # 25 transpose groups of 4 t's

        for r0, P in ROW_TILES:
            lam_sb = big_pool.tile([P, K * T], FP32, tag="lam")
            nc.sync.dma_start(lam_sb[:], lam[r0:r0 + P, :])
            et_sb = small_pool.tile([P, D], FP32, tag="et")
            nc.gpsimd.dma_start(et_sb[:], etf[r0:r0 + P, :])

            # softmax over K in fp32 (no max-subtract: |lambda| < ~0.7)
            th_sb = big_pool.tile([P, K * T], FP32, tag="th")
            nc.scalar.activation(th_sb[:], lam_sb[:], AF.Exp)
            sum_sb = small_pool.tile([P, T], FP32, tag="sum")
            nc.vector.tensor_reduce(
                out=sum_sb[:],
                in_=th_sb[:].rearrange("p (k t) -> p t k", k=K),
                axis=mybir.AxisListType.X, op=OP.add)
            rcp_sb = small_pool.tile([P, T], FP32, tag="rcp")
            nc.vector.reciprocal(rcp_sb[:], sum_sb[:])
            rcp_b = rcp_sb[:].rearrange("p (a t) -> p a t", a=1).to_broadcast((P, K, T))
            th_v3 = th_sb[:].rearrange("p (k t) -> p k t", k=K)
            nc.vector.tensor_tensor(out=th_v3, in0=th_v3, in1=rcp_b, op=OP.mult)
            nc.sync.dma_start(th_o[r0:r0 + P, :], th_sb[:])

            # bf16 copy of theta into the persistent padded staging tile
            nc.scalar.copy(th_bf[:P, :K * T], th_sb[:])

            # mask[n,d,t] = (ET >= t) in bf16
            mask_sb = big_pool.tile([P, D * T], BF16, tag="mask")
            et_b = et_sb[:].rearrange("p (d a) -> p d a", a=1).to_broadcast((P, D, T))
            iota_b = iota[:P].rearrange("p (a t) -> p a t", a=1).to_broadcast((P, D, T))
            nc.vector.tensor_tensor(
                out=mask_sb[:].rearrange("p (d t) -> p d t", d=D),
                in0=et_b, in1=iota_b, op=OP.is_ge)

            # phase 1: theta^T via matmul-against-identity into psum quadrants
            thT_all = big_pool.tile([128, NG * 128], BF16, tag="thT")
            for g in range(NG):
                thT_ps = psum_t.tile([128, P], FP32, tag="thT_ps")
                for tt in range(4):
                    t = 4 * g + tt
                    lhsT = bass.AP(
                        tensor=th_bf.tensor, offset=th_bf[:].offset + t,
                        ap=[[th_bf[:].ap[0][0], P], [T, KP]])
                    nc.tensor.matmul(
                        thT_ps[tt * KP:(tt + 1) * KP, :],
                        lhsT=lhsT, rhs=ident[:P, :P],
                        start=True, stop=True,
                        tile_position=(0, tt * KP))
                nc.scalar.copy(thT_all[:, g * 128:g * 128 + P], thT_ps[:])

            # phase 2: row-tiled matmuls; quadrant q handles t % 4 == q
            pi_sb = big_pool.tile([P, D * T], FP32, tag="pi")
            pp_v = pp_bf[:].rearrange("p (d t) -> p t d", d=D)   # [128, T, D]
            SG = 16
            t0 = 0
            while t0 < T:
                ntl = min(SG, T - t0) // 4
                for q in range(4):
                    pi_ps = psum_m.tile([P, ntl * D], FP32, tag=f"pi{q}")
                    for tl in range(ntl):
                        t = t0 + q + 4 * tl
                        g = t // 4
                        nc.tensor.matmul(
                            pi_ps[:, tl * D:(tl + 1) * D],
                            lhsT=thT_all[q * KP:q * KP + K, g * 128:g * 128 + P],
                            rhs=pp_v[q * KP:q * KP + K, t, :],
                            start=True, stop=True,
                            tile_position=(q * KP, 0))
                    out_ap = bass.AP(
                        tensor=pi_sb.tensor, offset=pi_sb[:].offset + t0 + q,
                        ap=[pi_sb[:].ap[0], [4, ntl], [T, D]])
                    mk_ap = bass.AP(
                        tensor=mask_sb.tensor, offset=mask_sb[:].offset + t0 + q,
                        ap=[mask_sb[:].ap[0], [4, ntl], [T, D]])
                    nc.vector.scalar_tensor_tensor(
                        out=out_ap,
                        in0=pi_ps[:].rearrange("p (tl d) -> p tl d", tl=ntl),
                        scalar=1.0 - EPS, in1=mk_ap,
                        op0=OP.min, op1=OP.mult)
                t0 += SG

            nc.vector.tensor_scalar_max(out=pi_sb[:], in0=pi_sb[:], scalar1=EPS)
            nc.sync.dma_start(pi_o[r0:r0 + P, :], pi_sb[:])

    nc.compile()
    return nc


_NC_CACHE = None


def kernel(**inputs):
    global LAST_RESULTS, _NC_CACHE
    lam = np.ascontiguousarray(np.asarray(inputs["lambda_"], dtype=np.float32))
    phi = np.ascontiguousarray(np.asarray(inputs["phi"], dtype=np.float32))
    et = np.asarray(inputs["event_times"])
    etf = np.ascontiguousarray(et.astype(np.float32))

    lam_sh = lam.reshape(NCORES, NS, K * T)
    etf_sh = etf.reshape(NCORES, NS, D)
    phi_flat = phi.reshape(K, D * T)

    if _NC_CACHE is None:
        _NC_CACHE = _build_nc()
    nc = _NC_CACHE

    in_maps = [
        {"lam": lam_sh[c], "phi": phi_flat, "etf": etf_sh[c]}
        for c in range(NCORES)
    ]
    res = run_bass_kernel_spmd(nc, in_maps, core_ids=list(range(NCORES)))
    LAST_RESULTS = res

    pi = np.concatenate([res.results[c]["pi"] for c in range(NCORES)], axis=0)
    th = np.concatenate([res.results[c]["theta"] for c in range(NCORES)], axis=0)
    pp = res.results[0]["phiprob"]
    return (
        pi.reshape(N, D, T),
        th.reshape(N, K, T),
        pp.reshape(K, D, T),
    )
